# revision 33
# baseline (speedup 1.0000x reference)
"""Trainium2 Bass kernel for nn_BiasAttention (B=8, N=2048, C=256, H=8).

Sharding: data-parallel over batch B across the 8 NeuronCores (one batch
element per core).  Weights and atten_bias are replicated to every core.

Per-core dataflow (everything kept "transposed" so the contraction dim is
always on SBUF partitions):
  xT   = x^T                              [C, N]     (PE transpose)
  qT,kT = wqkv rows 0..511 @ xT           [256, N]   (heads h at partitions 32h)
  v    = x @ wv^T                         [N, 256]   (natural, lhsT for attnv)
  sigT = sigmoid(atten_bias)^T            [N, N]     bf16, PE transpose + ACT
  per head-group hg (4 heads) and query block nb (512):
    S^T[m,n] = kT.T @ qT      4 heads concurrently via row-packed K=32 matmuls
    P = S^T * sigT            DVE tensor_tensor straight from PSUM -> bf16
    E = exp(P / sqrt(D))      ACT, in place
    outT[d,n] += v^T E        4 heads via col-packed M=32 matmuls
    Z[n]    += ones^T E       col-packed M=1 matmuls (softmax denominator)
  out = outT / Z              (reciprocal + partition-broadcast DMA)
  yT = wproj^T @ out          then + b_proj, PE transpose, DMA out
"""

import math

import numpy as np

B, N, C, H = 8, 2048, 256, 8
D = C // H  # 32
NCORES = 8
HG = 2  # head groups of 4
NB = N // 512  # 4 query blocks
MT = N // 128  # 16 key tiles

_cache = {}


def _build_module(reps=1, mode="full"):
    import concourse.bacc as bacc
    import concourse.mybir as mybir
    import concourse.tile as tile
    from concourse.bass import ds, ts
    from concourse.masks import make_identity

    f32 = mybir.dt.float32
    bf16 = mybir.dt.bfloat16
    AF = mybir.ActivationFunctionType
    MUL = mybir.AluOpType.mult

    nc = bacc.Bacc("TRN2", target_bir_lowering=False, debug=False,
                   num_devices=NCORES)

    x_d = nc.dram_tensor("x", [N, C], f32, kind="ExternalInput")
    bias_d = nc.dram_tensor("atten_bias", [N, N], f32, kind="ExternalInput")
    wqkv_d = nc.dram_tensor("w_qkv", [3 * C, C], f32, kind="ExternalInput")
    wproj_d = nc.dram_tensor("w_proj", [C, C], f32, kind="ExternalInput")
    bproj_d = nc.dram_tensor("b_proj", [C], f32, kind="ExternalInput")
    y_d = nc.dram_tensor("y", [N, C], f32, kind="ExternalOutput")

    with tile.TileContext(nc) as tc:
      for _rep in range(reps):
            with (
                tc.tile_pool(name="const", bufs=1) as const,
                tc.tile_pool(name="big", bufs=1) as big,
                tc.tile_pool(name="epool", bufs=3) as epool,
                tc.tile_pool(name="otn", bufs=1) as otn,
                tc.tile_pool(name="zstage", bufs=2) as zstage,
                tc.tile_pool(name="rzpool", bufs=2) as rzpool,
                tc.tile_pool(name="ytpool", bufs=2) as ytpool,
                tc.tile_pool(name="ystage", bufs=3) as ystage,
                tc.tile_pool(name="bstage", bufs=3) as bstage,
                tc.tile_pool(name="dpool", bufs=2, space="DRAM") as dpool,
                tc.tile_pool(name="aux", bufs=2, space="PSUM") as aux,
                tc.tile_pool(name="scps", bufs=2, space="PSUM") as scps,
                tc.tile_pool(name="outps", bufs=2, space="PSUM") as outps,
            ):
                ident = const.tile([128, 128], f32)
                make_identity(nc, ident)
                ones_col = const.tile([128, 1], bf16)
                nc.vector.memset(ones_col, 1.0)
                bproj_sb = const.tile([128, 2], f32)
                nc.sync.dma_start(bproj_sb, bproj_d[:].rearrange("(j p) -> p j", p=128))

                wqkvT = const.tile([128, 2, 768], bf16)   # [c, cc, o]
                wprojT = const.tile([128, 2, 256], bf16)  # [c, cc, j]
                wprojP = const.tile([128, 4, 256], bf16)  # permuted for aug layout
                qT = big.tile([128, HG, N], bf16)         # [32h+d, hg, n]
                kT = big.tile([128, HG, N], bf16)
                v_aug = big.tile([128, MT, 8, 64], bf16)  # [m, mt, h, (d|ones)]
                sigT = big.tile([128, MT, N], bf16)       # [m, mt, n]
                outTn = otn.tile([128, 4, N], bf16)       # [aug-c, 2hg+b, n]

                # ---------------- P0: weights + x transpose + qkv ----------------
                with tc.tile_pool(name="stage", bufs=4) as stage, \
                     tc.tile_pool(name="xtp", bufs=1) as xtp:
                    xT = xtp.tile([128, 2, N], bf16)      # [c, cc, n]
                    # w_qkv^T and w_proj^T
                    for wt, (wd, rows) in enumerate([(wqkv_d, 6), (wproj_d, 2)]):
                        dest = wqkvT if wt == 0 else wprojT
                        for ot in range(rows):
                            wst = stage.tile([128, 256], f32, tag="wst")
                            nc.sync.dma_start(wst, wd[ts(ot, 128), :])
                            pst = aux.tile([128, 512], f32, tag="aux")
                            for cc in range(2):
                                nc.tensor.transpose(pst[:, ts(cc, 128)],
                                                    wst[:, ts(cc, 128)], ident)
                            for cc in range(2):
                                nc.vector.tensor_copy(dest[:, cc, ts(ot, 128)],
                                                      pst[:, ts(cc, 128)])
                    # x^T
                    for nt in range(MT):
                        xst = stage.tile([128, 256], f32, tag="xst")
                        nc.sync.dma_start(xst, x_d[ts(nt, 128), :])
                        pst = aux.tile([128, 512], f32, tag="aux")
                        for cc in range(2):
                            nc.tensor.transpose(pst[:, ts(cc, 128)],
                                                xst[:, ts(cc, 128)], ident)
                        for cc in range(2):
                            nc.vector.tensor_copy(xT[:, cc, ts(nt, 128)],
                                                  pst[:, ts(cc, 128)])
                    # qT, kT  (o tiles 0,1 -> q ; 2,3 -> k); hg0 first so
                    # attention can start before hg1 weights are projected
                    for og in [0, 2, 1, 3]:
                        dest = qT if og < 2 else kT
                        hg = og % 2
                        for nb in range(NB):
                            ps = aux.tile([128, 512], f32, tag="aux")
                            for cc in range(2):
                                nc.tensor.matmul(ps, wqkvT[:, cc, ts(og, 128)],
                                                 xT[:, cc, ts(nb, 512)],
                                                 start=(cc == 0), stop=(cc == 1))
                            nc.vector.tensor_copy(dest[:, hg, ts(nb, 512)], ps)
                    # v (natural layout), interleaved with a ones column block
                    # per head: lhsT = [v_h | ones] gives out and Z in one matmul
                    nc.vector.memset(v_aug[:, :, :, 32:64], 1.0)
                    for mt in range(MT):
                        ps = aux.tile([128, 512], f32, tag="aux")
                        for cc in range(2):
                            nc.tensor.matmul(ps[:, :256], xT[:, cc, ts(mt, 128)],
                                             wqkvT[:, cc, 512:768],
                                             start=(cc == 0), stop=(cc == 1))
                        nc.vector.tensor_copy(
                            v_aug[:, mt, :, 0:32],
                            ps[:, :256].rearrange("p (h d) -> p h d", h=8))
                    # permuted w_proj^T matching the [out|Z] interleaved layout:
                    # chunk cc2 = 2*hg + b holds head (4hg+2b) at rows 0-31 and
                    # head (4hg+2b+1) at rows 64-95; Z rows get zero weights.
                    nc.vector.memset(wprojP, 0.0)
                    for hg in range(HG):
                        for b in range(2):
                            nc.sync.dma_start(wprojP[0:32, 2 * hg + b, :],
                                              wprojT[64 * b:64 * b + 32, hg, :])
                            nc.sync.dma_start(wprojP[64:96, 2 * hg + b, :],
                                              wprojT[64 * b + 32:64 * b + 64, hg, :])


                # ---------------- P1+P2: attention, bias sigmoid JIT -------------
                def bias_block(nbi):
                    # produce sigT rows for bias rows [512*nbi, 512*nbi+512)
                    for nt in range(4 * nbi, 4 * nbi + 4):
                        for mq in range(4):
                            bst = bstage.tile([128, 512], f32, tag="bst")
                            nc.sync.dma_start(
                                bst, bias_d[ts(nt, 128), ts(mq, 512)])
                            pst = aux.tile([128, 512], f32, tag="aux")
                            for j in range(4):
                                nc.tensor.transpose(
                                    pst[:, ts(j, 128)],
                                    bst[:, ts(j, 128)], ident)
                            sg = sigT[:, mq * 4:(mq + 1) * 4, ts(nt, 128)]
                            nc.scalar.activation(
                                sg, pst.rearrange("p (j f) -> p j f", j=4),
                                AF.Tanh, scale=0.5)
                            nc.gpsimd.tensor_scalar(
                                sg, sg, 0.5, 0.5,
                                mybir.AluOpType.mult, mybir.AluOpType.add)

                scale = float(D ** -0.5)
                LAG = 8  # j-steps between scores+mul and the matching attnv
                pending = []

                def drain(limit):
                    while len(pending) > limit:
                        pending.pop(0)()

                do_mul = mode != "scores"
                do_attnv = mode in ("full", "noexp")
                do_exp = mode in ("full",)
                for _nbi in range(NB):
                    bias_block(_nbi)
                for hg in range(HG):
                    for nb in range(NB):
                        out_ab = [outps.tile([128, 512], f32, tag="o",
                                             name=f"oab{hg}{nb}{b}")
                                  for b in range(2)]
                        e_ts = [None] * 4

                        def attnv(mq, j, hg=hg, nb=nb, out_ab=out_ab, e_ts=e_ts):
                            mt = mq * 4 + j
                            first = mt == 0
                            last = mt == MT - 1
                            e_t = e_ts[mq % 4]
                            for h in range(4):
                                nc.tensor.matmul(
                                    out_ab[h // 2][64 * (h % 2):64 * (h % 2) + 64, :],
                                    v_aug[:, mt, hg * 4 + h, :],
                                    e_t[:, j, h],
                                    start=first, stop=last,
                                    tile_position=(0, 64 * (h % 2)))

                        def tail(hg=hg, nb=nb, out_ab=out_ab):
                            for b in range(2):
                                st = zstage.tile([128, 512], f32, tag="zst")
                                nc.scalar.copy(st, out_ab[b])
                                rz_st = zstage.tile([128, 512], f32, tag="rzst")
                                nc.vector.reciprocal(rz_st, st)
                                rz_dr = dpool.tile([2, 512], f32, tag="rzd")
                                nc.sync.dma_start(rz_dr, rz_st[32:97:64, :])
                                rz_bc = rzpool.tile([128, 512], f32, tag="rz")
                                for r in range(2):
                                    nc.sync.dma_start(
                                        rz_bc[64 * r:64 * r + 64, :],
                                        rz_dr[r:r + 1, :].to_broadcast((64, 512)))
                                nc.gpsimd.tensor_tensor(
                                    outTn[:, 2 * hg + b, ts(nb, 512)], st, rz_bc,
                                    MUL)
                            if hg == 1:
                                yts = []
                                for jt in range(2):
                                    pp = aux.tile([128, 512], f32, tag="aux")
                                    for cc2 in range(4):
                                        nc.tensor.matmul(
                                            pp, wprojP[:, cc2, ts(jt, 128)],
                                            outTn[:, cc2, ts(nb, 512)],
                                            start=(cc2 == 0), stop=(cc2 == 3))
                                    yt = ytpool.tile([128, 512], f32, tag="yt")
                                    nc.vector.tensor_scalar_add(
                                        yt, pp, bproj_sb[:, jt:jt + 1])
                                    yts.append(yt)
                                for k in range(4):
                                    nt = nb * 4 + k
                                    yo = aux.tile([128, 512], f32, tag="aux")
                                    for jt in range(2):
                                        nc.tensor.transpose(
                                            yo[:, ts(jt, 128)],
                                            yts[jt][:, ts(k, 128)], ident)
                                    y_st = ystage.tile([128, 256], f32, tag="yst")
                                    nc.scalar.copy(y_st, yo[:, :256])
                                    nc.sync.dma_start(y_d[ts(nt, 128), :], y_st)

                        for mq in range(4):
                            e_t = epool.tile([128, 4, 4, 512], bf16, tag="e")
                            e_ts[mq % 4] = e_t
                            for j in range(4):
                                mt = mq * 4 + j
                                for hp in range(2):
                                    sc = scps.tile([128, 2, 512], f32, tag="s")
                                    for hh in range(2):
                                        h = hp * 2 + hh
                                        nc.tensor.matmul(
                                            sc[:, hh, :],
                                            kT[32 * h:32 * (h + 1), hg, ts(mt, 128)],
                                            qT[32 * h:32 * (h + 1), hg, ts(nb, 512)],
                                            start=True, stop=True,
                                            tile_position=(32 * h, 0))
                                    if do_mul:
                                        sig_bc = sigT[:, mt:mt + 1,
                                                      ts(nb, 512)].to_broadcast(
                                                          (128, 2, 512))
                                        dst = e_t[:, j, 2 * hp:2 * hp + 2]
                                        nc.vector.tensor_tensor(
                                            dst, sc, sig_bc, MUL)
                                    else:
                                        nc.vector.tensor_copy(
                                            e_t[:, j, 2 * hp:2 * hp + 2], sc)
                                if do_attnv:
                                    pending.append(
                                        (lambda mq=mq, j=j, fn=attnv:
                                         fn(mq, j)))
                                    drain(LAG)
                            if do_exp:
                                nc.scalar.activation(e_t, e_t, AF.Exp, scale=scale)
                        if do_attnv:
                            pending.append(tail)
                drain(0)

    nc.compile()
    return nc


def _get_module():
    if "nc" not in _cache:
        _cache["nc"] = _build_module()
    return _cache["nc"]


class _Runner:
    """Persistent jitted shard_map executor (mirrors bass2jax.run_bass_via_pjrt
    but keeps one jit cache entry so repeated calls don't recompile)."""

    def __init__(self, nc):
        import jax
        from jax.experimental.shard_map import shard_map
        from jax.sharding import Mesh, NamedSharding, PartitionSpec

        import concourse.mybir as mybir
        from concourse import bass2jax

        bass2jax.install_neuronx_cc_hook()
        assert nc.dbg_addr is None
        partition_name = (nc.partition_id_tensor.name
                          if nc.partition_id_tensor else None)
        in_names, out_names, out_avals, zero_outs = [], [], [], []
        for alloc in nc.m.functions[0].allocations:
            if not isinstance(alloc, mybir.MemoryLocationSet):
                continue
            name = alloc.memorylocations[0].name
            if alloc.kind == "ExternalInput":
                if name != partition_name:
                    in_names.append(name)
            elif alloc.kind == "ExternalOutput":
                out_names.append(name)
                shape = tuple(alloc.tensor_shape)
                dtype = mybir.dt.np(alloc.dtype)
                out_avals.append(jax.core.ShapedArray(shape, dtype))
                zero_outs.append(np.zeros(shape, dtype))
        self.in_names = in_names
        self.out_names = out_names
        self.out_avals = out_avals
        all_in = tuple(in_names) + tuple(out_names)
        if partition_name is not None:
            all_in = all_in + (partition_name,)

        def _body(*args):
            operands = list(args)
            if partition_name is not None:
                operands.append(bass2jax.partition_id_tensor())
            outs = bass2jax._bass_exec_p.bind(
                *operands,
                out_avals=tuple(out_avals),
                in_names=all_in,
                out_names=tuple(out_names),
                lowering_input_output_aliases=(),
                sim_require_finite=True,
                sim_require_nnan=True,
                nc=nc,
            )
            return tuple(outs)

        devices = jax.devices()[:NCORES]
        mesh = Mesh(np.asarray(devices), ("core",))
        nspec = len(in_names) + len(out_names)
        self._fn = jax.jit(
            shard_map(_body, mesh=mesh,
                      in_specs=(PartitionSpec("core"),) * nspec,
                      out_specs=(PartitionSpec("core"),) * len(out_names),
                      check_rep=False),
            keep_unused=True)
        self._sharding = NamedSharding(mesh, PartitionSpec("core"))
        self._jax = jax
        self._zero_dev = [
            jax.device_put(np.concatenate([z] * NCORES, axis=0), self._sharding)
            for z in zero_outs
        ]

    def put_inputs(self, in_maps):
        concat = [
            np.concatenate([np.asarray(m[nm]) for m in in_maps], axis=0)
            for nm in self.in_names
        ]
        return [self._jax.device_put(a, self._sharding) for a in concat]

    def run(self, dev_inputs):
        outs = self._fn(*dev_inputs, *self._zero_dev)
        self._jax.block_until_ready(outs)
        return outs


def _get_runner():
    if "runner" not in _cache:
        _cache["runner"] = _Runner(_get_module())
    return _cache["runner"]


def _make_in_maps(x, atten_bias, w_qkv, w_proj, b_proj):
    x = np.asarray(x, dtype=np.float32)
    atten_bias = np.ascontiguousarray(np.asarray(atten_bias, dtype=np.float32))
    w_qkv = np.ascontiguousarray(np.asarray(w_qkv, dtype=np.float32))
    w_proj = np.ascontiguousarray(np.asarray(w_proj, dtype=np.float32))
    b_proj = np.ascontiguousarray(np.asarray(b_proj, dtype=np.float32))
    return [
        {
            "x": np.ascontiguousarray(x[b]),
            "atten_bias": atten_bias,
            "w_qkv": w_qkv,
            "w_proj": w_proj,
            "b_proj": b_proj,
        }
        for b in range(B)
    ]


def kernel(x, atten_bias, w_qkv, w_proj, b_proj):
    runner = _get_runner()
    in_maps = _make_in_maps(x, atten_bias, w_qkv, w_proj, b_proj)
    dev = runner.put_inputs(in_maps)
    outs = runner.run(dev)
    y = np.asarray(outs[runner.out_names.index("y")])
    return y.reshape(B, N, C).astype(np.float32)
